# revision 58
# baseline (speedup 1.0000x reference)
"""Dense transformer block (nn_Block_87127706566879) on 8 TRN2 NeuronCores.

Sharding: DP over batch (4 pairs) x TP=2 within each pair.
Attention is head-split (8 of 16 heads per core, Megatron-style); the MLP
is TOKEN-split (each core owns half the sequence). The single collective
per core pair is a ReduceScatter after the attention out-proj: it sums
the two cores' partial out-proj contributions AND scatters the residual
stream by token ownership in one shot. The MLP then runs fully local on
the owned tokens (full FF width) and the block output needs no second
collective. The ReduceScatter is split into two token-quarter chunks so
the first can overlap the tail of attention + out-proj compute.

The residual stream lives TRANSPOSED on chip ([E, S], e on partitions).
LayerNorm stats (over e = partition dim) are computed with ones-vector
matmuls on the PE; per-s stats are broadcast back across partitions with
a K=1 ones matmul. LN gains/biases are folded into the following matmul
weights host-side; x is pre-scaled by 0.5 so the pair ReduceScatter
reconstructs x + attn exactly.

Matmuls run in float32r (fp32 storage, ~tf32 precision, full PE rate at
free-dim >= 256). Attention: scores are computed transposed
(S^T = [k, q], keys stationary, two heads row-packed via tile_position);
softmax exp runs on ACT with the periodic (k % td == td-1) mask folded
into the per-partition bias and the 1/sqrt(dh) scale folded into the
activation scale; causal masking of diagonal tiles is a 0/1 mask
multiply on the DVE. The AV matmul appends a ones column to V
([V | 1], M=65) so the softmax denominator accumulates for free in PSUM
row 64; normalization happens once on the (much smaller) attn output.
The MLP hidden activations are kept in bf16 (gelu writes bf16 directly)
so the full-FF hidden fits in SBUF; fc2 weights are bf16 as well.

Scheduling (what got HW exec from 919us to ~730us, neuron-profile):
- The attention inner loop is software-pipelined: AV matmuls lag the
  score matmuls by 2 k-tiles so the PE never blocks on the ACT exp, and
  each head-pair's epilogue is split into an early PSUM-release (copies
  of the unnormalized attn rows + raw denominators) dropped into the
  next head-pair's pipeline, plus a slack-tolerant normalization.
- The attention phase is ACT(exp)-bound, so the next chunk's LN1+QKV
  (and ready out-proj quarters) are emitted as per-slot fillers inside
  the attention pipeline -- the PE stream stays dense, which also keeps
  the HAM clock-gate at K=8/8 (cold-clock time fell 459us -> ~180us).
  A ~10us burst of dummy K=1 matmuls at kernel start pre-warms HAM
  during the initial weight DMAs.
- 1/x and rsqrt run on ACT as exp(-ln(x)) / exp(-0.5 ln(x)): ln+exp
  share one act-table set with the attention exp (bass's act-table
  chooser is patched accordingly), replacing 3.3us-each [1,512] DVE
  reciprocals and 40+ act-table thrashes.
- The ReduceScatters run in bf16 (residual error ~4e-3 << the 2e-2
  gate) and the RS-dependent x2 loads go through SWDGE (gpsimd) so a
  blocked collective wait never heads the SP HWDGE ring that streams
  the fc1/fc2 weights.
- fp8(e4m3) DoubleRow for the MLP was tried and REJECTED: it measures
  1.6-2.7e-2 rel max-err on the real inputs -- no margin at tol 2e-2.
"""

import itertools
import math
from contextlib import ExitStack
from dataclasses import dataclass

import numpy as np

import concourse.bass as bass
import concourse.tile as tile
from concourse import bacc, mybir
from concourse._compat import with_exitstack

F32 = mybir.dt.float32
F32R = mybir.dt.float32r
BF16 = mybir.dt.bfloat16
AF = mybir.ActivationFunctionType
OP = mybir.AluOpType
NEG = -1e30


_ACT_TABLES_PATCHED = False


def _patch_act_tables():
    """Restrict the act-table chooser to one exp+ln set (plus gelu/sigmoid).

    bacc's insert_act_table_loads picks the first act_info.json set that
    contains each function: Exp -> 'exp_and_others', Ln -> 'natural_log',
    which alternate every softmax-normalize (47 x 1.28us table loads on
    HW). Emptying all sets except 'natural_log_exp_and_others' (which has
    BOTH exp and ln), 'gelu_and_others', and 'sigmoid_and_others' forces a
    thrash-free choice; indices stay aligned with act_info.json.
    """
    global _ACT_TABLES_PATCHED
    if _ACT_TABLES_PATCHED:
        return
    _ACT_TABLES_PATCHED = True
    import functools
    import concourse.hw_specs as _hs
    import concourse.bacc as _bc

    orig = _hs.get_activation_tables
    keep = {"natural_log_exp_and_others", "gelu_and_others", "sigmoid_and_others"}

    @functools.cache
    def patched(arch):
        return {k: (v if k in keep else set()) for k, v in orig(arch).items()}

    _hs.get_activation_tables = patched
    _bc.get_activation_tables = patched


def act_fn(nc, out, in_, func, bias=0.0, scale=1.0):
    """nc.scalar.activation clone that also permits Reciprocal/Rsqrt.

    bass blocks these two on ACT for accuracy reasons; at this kernel's
    2e-2 tolerance the LUT error (~1e-3) is fine and it avoids the very
    slow [1, n] DVE reciprocal (3.3us each on HW)."""
    s = nc.scalar
    ins = [s.lower_ap(in_)]
    if func in (AF.Copy, AF.Reciprocal):
        bias_arg = mybir.ImmediateValue(dtype=mybir.dt.float32, value=float(bias))
    else:
        bias_arg = s.lower_ap(s.bass.const_aps.scalar_like(float(bias), in_))
    ins.append(bias_arg)
    ins.append(mybir.ImmediateValue(dtype=mybir.dt.float32, value=float(scale)))
    ins.append(mybir.ImmediateValue(dtype=mybir.dt.float32, value=0.0))
    return s.add_instruction(
        mybir.InstActivation(
            name=s.bass.get_next_instruction_name(),
            func=func,
            ins=ins,
            outs=[s.lower_ap(out)],
        )
    )


@dataclass(frozen=True)
class Cfg:
    B: int = 4
    S: int = 2048
    E: int = 1024
    H: int = 16
    FF: int = 4096
    n_pairs: int = 4  # cores = 2 * n_pairs
    reps: int = 1
    rsb_wait_ms: float = 0.5  # virtual-time hint for RS_b-dependent ops
    gelu_exact: bool = True  # False: x*sigmoid(1.702x) (CoreSim lacks Gelu)
    no_collective: bool = False  # replace RS with local DMA (timeline sim only)

    @property
    def Dh(self):
        return self.E // self.H

    @property
    def HL(self):
        return self.H // 2  # heads per core

    @property
    def HP(self):
        return self.HL // 2  # head pairs per core

    @property
    def OL(self):
        return self.HL * self.Dh  # attn out dims per core

    @property
    def ET(self):
        return self.E // 128

    @property
    def ST(self):
        return self.S // 128

    @property
    def SH(self):
        return self.S // 2  # tokens owned per core (MLP)

    @property
    def SC(self):
        return self.SH // 512 if self.SH >= 512 else 1  # 512-chunks of owned tokens

    @property
    def CW(self):
        return min(self.SH, 512)  # chunk width in MLP

    @property
    def QW(self):
        return self.S // 4  # token quarter (ReduceScatter granularity)

    @property
    def C4(self):
        return self.S // 512  # q-chunks (attention / phase-1)

    @property
    def FT(self):
        return self.FF // 128  # full-FF tiles (MLP is token-split)

    @property
    def OT(self):
        return self.OL // 128  # attn o-tiles (= head pairs)


@with_exitstack
def block_kernel(ctx: ExitStack, tc: tile.TileContext, cfg: Cfg, ins, outs):
    nc = tc.nc
    ET, SC, C4, FT, HP, OT = cfg.ET, cfg.SC, cfg.C4, cfg.FT, cfg.HP, cfg.OT
    S, E, OL, QW, CW = cfg.S, cfg.E, cfg.OL, cfg.QW, cfg.CW
    ST = cfg.ST
    eps1 = 1e-5 / 4.0  # LN1 runs on x/2
    eps2 = 1e-5       # LN2 runs on exact x2
    groups = [[2 * p, 2 * p + 1] for p in range(cfg.n_pairs)]

    cst = ctx.enter_context(tc.tile_pool(name="cst", bufs=1))

    def load_const(name, dt_):
        t = cst.tile(list(ins[name].shape), dt_, tag=name)
        nc.sync.dma_start(t[:], ins[name])
        return t

    onesrow = load_const("onesrow", F32R)      # [1, 512]
    onehot2 = load_const("onehot2", F32R)      # [2, 128]
    ones128 = load_const("ones128", F32R)      # [128, 8] (col 0: stats lhsT)
    ones128b = load_const("ones128b", BF16)    # [128, 1] bf16 stats lhsT
    maskd = load_const("maskd", BF16)          # [128, 128] 0/1 lower-tri (k<=q)
    pbias = load_const("pbias", F32)           # [128, ST] periodic -1e30 bias
    bqk = load_const("bqk", F32)               # [128, 2*OT]
    bv = load_const("bv", F32R)                # [1, OL]
    bfc1 = load_const("bfc1", F32)             # [128, FT]

    def ln_stats_apply(pools, x_tiles, sc_off, eps, out_tiles, out_off, n):
        """LN over the partition (e) dim for one n-col chunk of x^T."""
        ps_stat, ps_bc, sb_small, sb_big, sb_bc = pools
        stat = ps_stat.tile([64, n], F32, tag="stat", name="stat")
        psum_s = stat[0:1, :]
        psum_q = stat[32:33, :]
        for et in range(ET):
            xr = x_tiles[et][:, sc_off : sc_off + n]
            is16 = xr.dtype == BF16
            xr_f = xr if xr.dtype in (F32, BF16) else xr.bitcast(F32)
            if is16:
                xr_r = xr
            else:
                xr_r = xr if xr.dtype == F32R else xr.bitcast(F32R)
            sq = sb_big.tile([128, n], BF16, tag="sqt", name="sqt")
            nc.scalar.square(sq[:], xr_f)
            nc.tensor.matmul(
                psum_s[:],
                lhsT=(ones128b[:, 0:1] if is16 else ones128[:, 0:1]),
                rhs=xr_r,
                start=(et == 0), stop=(et == ET - 1),
            )
            nc.tensor.matmul(
                psum_q[:], lhsT=ones128b[:, 0:1], rhs=sq[:],
                start=(et == 0), stop=(et == ET - 1),
            )
        inv_e = 1.0 / cfg.E
        m = sb_small.tile([1, n], F32R, tag="m", name="m")
        nc.vector.tensor_scalar(m[:], psum_s[:], inv_e, None, OP.mult)
        var = sb_small.tile([1, n], F32, tag="var", name="var")
        nc.vector.tensor_scalar(var[:], psum_q[:], inv_e, eps, OP.mult, OP.add)
        t1 = sb_small.tile([1, n], F32, tag="t1", name="t1")
        nc.vector.tensor_mul(t1[:], m[:].bitcast(F32), m[:].bitcast(F32))
        nc.vector.tensor_sub(var[:], var[:], t1[:])
        # rstd = exp(-0.5 ln(var+eps)) on ACT: ln and exp share one act
        # table set (natural_log_exp_and_others) with the attention exp,
        # so this neither thrashes the 1.28us act tables nor pays the
        # 3.3us [1, n] DVE reciprocal.
        lg = sb_small.tile([1, n], F32, tag="lg", name="lg")
        act_fn(nc, lg[:], var[:], AF.Ln)
        rstd = sb_small.tile([1, n], F32R, tag="rstd", name="rstd")
        act_fn(nc, rstd[:], lg[:], AF.Exp, scale=-0.5)
        # broadcast m, rstd across partitions via K=1 ones matmul
        pm = ps_bc.tile([128, n], F32, tag="a", name="pm")
        nc.tensor.matmul(pm[:], lhsT=onesrow[:, 0:128], rhs=m[:], start=True, stop=True)
        m_b = sb_bc.tile([128, n], F32, tag="m_b", name="m_b")
        nc.vector.tensor_copy(m_b[:], pm[:])
        pv = ps_bc.tile([128, n], F32, tag="a", name="pv")
        nc.tensor.matmul(pv[:], lhsT=onesrow[:, 0:128], rhs=rstd[:], start=True, stop=True)
        r_b = sb_bc.tile([128, n], F32, tag="r_b", name="r_b")
        nc.vector.tensor_copy(r_b[:], pv[:])
        for et in range(ET):
            src_ap = x_tiles[et][:, sc_off : sc_off + n]
            src_f = src_ap if src_ap.dtype in (F32, BF16) else src_ap.bitcast(F32)
            out_ap = out_tiles[et][:, out_off : out_off + n]
            nc.vector.tensor_sub(out_ap, src_f, m_b[:])
            nc.vector.tensor_mul(out_ap, out_ap, r_b[:])

    for rep in range(cfg.reps):
        # -------- LN1 + QKV + attention (interleaved per 512-chunk) ------
        # QKV projection work is PE+DVE-heavy while attention is ACT-heavy
        # (softmax exp): interleaving the two per chunk overlaps them.
        # Attention for chunk c needs exactly the K/V tiles chunks <= c
        # produced. PSUM budget (8 banks): ps_a 2 (QK-proj / scores /
        # denominator-broadcast / out-proj, one rotating tag), ps_v 1,
        # ps_av 2, ps_stat 1 (sum at partition 0, sum-of-squares at 32),
        # ps_bb 2.
        rs_in = [
            nc.dram_tensor(f"rs_in{rep}_{g}", [2, E, QW], BF16) for g in range(2)
        ]
        rs_out = [
            nc.dram_tensor(f"rs_out{rep}_{g}", [E, QW], BF16) for g in range(2)
        ]
        qk_vo_stack = ExitStack()
        qk_pool = qk_vo_stack.enter_context(tc.tile_pool(name="qk", bufs=1))
        qkT = [qk_pool.tile([128, S], BF16, tag=f"qkT{i}", name=f"qkT{i}") for i in range(2 * OT)]
        vo_pool = qk_vo_stack.enter_context(tc.tile_pool(name="vo", bufs=1))
        VO = [vo_pool.tile([128, cfg.HL * 65], BF16, tag=f"vo{i}", name=f"vo{i}") for i in range(ST)]
        at_pool = qk_vo_stack.enter_context(tc.tile_pool(name="attnT", bufs=1))
        attnT = [at_pool.tile([128, S], BF16, tag=f"at{i}", name=f"at{i}") for i in range(OT)]
        ph_stack = ExitStack()
        _p = lambda *a, **k: ph_stack.enter_context(tc.tile_pool(*a, **k))
        xq_pool = _p(name="xq_sb", bufs=2)
        xn1_pool = _p(name="xn1_sb", bufs=2)
        wqk_pool = _p(name="wqk", bufs=1)
        wv_pool = _p(name="wv", bufs=1)
        wo_pool = _p(name="wo", bufs=1)
        pt_pool = _p(name="pt", bufs=3)
        den_pool = _p(name="den", bufs=2)
        xqs_pool = _p(name="xqs", bufs=2)
        arin_pool = _p(name="arin", bufs=2)
        # PSUM budget (8 banks): ps_a 2 (QKV / V / out-proj / K=1 broadcast,
        # one rotating tag), ps_sc 3 (bf16 score tiles, 1 bank each -> 3-deep
        # score lookahead ahead of exp), ps_av 2 (AV accumulator), ps_stat 1.
        ps_a = _p(name="ps_a", bufs=2, space="PSUM")
        ps_sc = _p(name="ps_sc", bufs=3, space="PSUM")
        ps_av = _p(name="ps_av", bufs=1, space="PSUM")
        ps_stat = _p(name="ps_stat", bufs=1, space="PSUM")
        sb_small = _p(name="sb_small", bufs=2)
        sb_big = _p(name="sb_big", bufs=2)
        sb_bc = _p(name="sb_bc", bufs=1)
        if True:
            ln_pools = (ps_stat, ps_a, sb_small, sb_big, sb_bc)
            scale = 1.0 / math.sqrt(cfg.Dh)
            # PE warm-up: ~10us of dummy K=1 matmuls (they only need the
            # tiny onesrow const) trip the HAM activity window to K=8/8
            # during the initial weight DMAs, so the first real matmuls run
            # at 2.4 GHz instead of paying the cold-clock ramp.
            for _ in range(30):
                wup = ps_sc.tile([128, 512], F32, tag="sc", name="wup")
                nc.tensor.matmul(
                    wup[:], lhsT=onesrow[:, 0:128], rhs=onesrow[:, 0:512],
                    start=True, stop=True,
                )
            # prefetch the first x chunk ahead of the (large) weight DMAs so
            # LN1 of chunk 0 starts immediately
            xq_first = [
                xq_pool.tile([128, 512], F32R, tag=f"xq{et}", name=f"xq{et}")
                for et in range(ET)
            ]
            for et in range(ET):
                nc.sync.dma_start(
                    xq_first[et][:], ins["xq"][et * 128 : (et + 1) * 128, 0:512]
                )
            wqk_sb = wqk_pool.tile([128, ET, 2 * OL], BF16, tag="wqk", name="wqk")
            nc.sync.dma_start(
                wqk_sb[:], ins["wqkT"].rearrange("(et p) o -> p et o", p=128)
            )
            wv_sb = wv_pool.tile([128, ET, OL], BF16, tag="wv", name="wv")
            nc.sync.dma_start(
                wv_sb[:], ins["wvT"].rearrange("(et p) o -> p et o", p=128)
            )
            wo_sb = wo_pool.tile([128, OT, E], BF16, tag="wo", name="wo")
            nc.sync.dma_start(wo_sb[:], ins["woT"].rearrange("(ot p) e -> p ot e", p=128))

            def outproj_steps(qq, slot):
                """Partial out-proj + (0.5x + 0.5bo) residual for quarter qq
                -> rs_in[slot] (the bias rides in xqr, host-folded), sliced
                per e-tile so it can fill attention-pipeline PE slack."""
                for et in range(ET):
                    w = min(QW, 512)
                    coff = qq * QW
                    psum = ps_a.tile([128, 512], F32, tag="a", name="o")
                    for ot in range(OT):
                        nc.tensor.matmul(
                            psum[:, 0:w],
                            lhsT=wo_sb[:, ot, et * 128 : (et + 1) * 128],
                            rhs=attnT[ot][:, coff : coff + w],
                            start=(ot == 0),
                            stop=(ot == OT - 1),
                        )
                    xqs = xqs_pool.tile([128, w], F32, tag="xqs", name="xqs")
                    nc.sync.dma_start(
                        xqs[:],
                        ins["xqr"][et * 128 : (et + 1) * 128, coff : coff + w],
                    )
                    arin = arin_pool.tile([128, w], BF16, tag="arin", name="arin")
                    nc.vector.tensor_add(arin[:], psum[:, 0:w], xqs[:])
                    g, member = slot // 2, slot % 2
                    nc.sync.dma_start(
                        rs_in[g][member, et * 128 : (et + 1) * 128, 0:w], arin[:]
                    )
                    yield

            def outproj_quarter(qq, slot):
                for _ in outproj_steps(qq, slot):
                    pass

            xq_of = {0: xq_first}
            xn1_of = {}

            def emit_ln1(ci):
                xn1c = [
                    xn1_pool.tile([128, 512], BF16, tag=f"xn1{et}", name=f"xn1{et}")
                    for et in range(ET)
                ]
                ln_stats_apply(ln_pools, xq_of.pop(ci), 0, eps1, xn1c, 0, 512)
                xn1_of[ci] = xn1c

            def make_tail(hp, c, av):
                """Per-head-pair epilogue, split so it can slot into the next
                head-pair's pipeline: `copies` frees the av PSUM bank (must
                run before the next AV chain starts); `norm` is the slack-
                tolerant 1/den = exp(-ln(den)) + broadcast + in-place mul."""
                dens = [
                    den_pool.tile([1, 512], F32, tag=f"den{h01}", name=f"den{h01}")
                    for h01 in range(2)
                ]

                def copies():
                    for h01 in range(2):
                        nc.vector.tensor_copy(dens[h01][:], av[h01][64:65, :])
                    for h01 in range(2):
                        nc.vector.tensor_copy(
                            attnT[hp][h01 * 64 : (h01 + 1) * 64, c * 512 : (c + 1) * 512],
                            av[h01][0:64, :],
                        )

                def norm():
                    for h01 in range(2):
                        lg = den_pool.tile([1, 512], F32, tag="lg", name="lg")
                        act_fn(nc, lg[:], dens[h01][:], AF.Ln)
                        rd = den_pool.tile([1, 512], F32R, tag="rd", name="rd")
                        with nc.allow_low_precision(reason="1/den f32r for matmul rhs"):
                            act_fn(nc, rd[:], lg[:], AF.Exp, scale=-1.0)
                        pbc = ps_a.tile([128, 512], F32, tag="a", name="bc")
                        nc.tensor.matmul(
                            pbc[:], lhsT=onesrow[:, 0:128], rhs=rd[:],
                            start=True, stop=True,
                        )
                        nc.vector.tensor_mul(
                            attnT[hp][h01 * 64 : (h01 + 1) * 64, c * 512 : (c + 1) * 512],
                            attnT[hp][h01 * 64 : (h01 + 1) * 64, c * 512 : (c + 1) * 512],
                            pbc[h01 * 64 : (h01 + 1) * 64, :],
                        )

                return copies, norm

            def qkv_steps(ci):
                """QKV projection for chunk ci, sliced into generator steps
                (~2-3 matmuls each) so it can interleave into the previous
                chunk's ACT-bound attention pipeline and keep the PE dense."""
                xn1c = xn1_of.pop(ci)
                co2 = ci * 512
                # Q,K projections: out qkT [o, s] (w stationary)
                for ot in range(2 * OT):
                    psum = ps_a.tile([128, 512], F32, tag="a", name="qk")
                    for et in range(ET):
                        nc.tensor.matmul(
                            psum[:],
                            lhsT=wqk_sb[:, et, ot * 128 : (ot + 1) * 128],
                            rhs=xn1c[et][:],
                            start=(et == 0),
                            stop=(et == ET - 1),
                        )
                        if et % 3 == 2:
                            yield
                    nc.vector.tensor_scalar(
                        qkT[ot][:, co2 : co2 + 512],
                        psum[:],
                        bqk[:, ot : ot + 1],
                        None,
                        OP.add,
                    )
                    yield
                # V projection: out V [s, o_v] (xn1 stationary), bias preloaded
                for stl in range(4):  # s-tiles within this 512-chunk
                    st = co2 // 128 + stl
                    psum = ps_a.tile([128, 512], F32, tag="a", name="v")[:, 0:OL]
                    nc.tensor.matmul(
                        psum[:, 0:OL], lhsT=onesrow[:, 0:128], rhs=bv[:],
                        start=True, stop=False,
                    )
                    for et in range(ET):
                        nc.tensor.matmul(
                            psum[:],
                            lhsT=xn1c[et][:, stl * 128 : (stl + 1) * 128],
                            rhs=wv_sb[:, et],
                            start=False,
                            stop=(et == ET - 1),
                        )
                        if et % 3 == 2:
                            yield
                    for h in range(cfg.HL):
                        nc.vector.tensor_copy(
                            VO[st][:, h * 65 : h * 65 + 64],
                            psum[:, h * 64 : (h + 1) * 64],
                        )
                    nc.vector.tensor_copy(
                        VO[st][:, 64 :: 65], ones128[:, 0 : cfg.HL]
                    )
                    yield

            emit_ln1(0)
            for _ in qkv_steps(0):
                pass
            if C4 > 1:
                xq_of[1] = [
                    xq_pool.tile([128, 512], F32R, tag=f"xq{et}", name=f"xq{et}")
                    for et in range(ET)
                ]
                for et in range(ET):
                    nc.sync.dma_start(
                        xq_of[1][et][:],
                        ins["xq"][et * 128 : (et + 1) * 128, 512:1024],
                    )
            for ch in range(C4):
                coff = ch * 512
                # prefetch x two chunks ahead (LN1(ch+1) is emitted mid-
                # attention below, so xq(ch+1) was fetched a chunk ago)
                if ch + 2 < C4:
                    xq_sb = [
                        xq_pool.tile([128, 512], F32R, tag=f"xq{et}", name=f"xq{et}")
                        for et in range(ET)
                    ]
                    for et in range(ET):
                        nc.sync.dma_start(
                            xq_sb[et][:],
                            ins["xq"][
                                et * 128 : (et + 1) * 128, coff + 1024 : coff + 1536
                            ],
                        )
                    xq_of[ch + 2] = xq_sb
                # ---- attention for q-chunk ch (software-pipelined) ----
                # The AV matmuls lag the score matmuls by LAG iterations so
                # the PE stream never waits on the ACT exp; the previous
                # head-pair's epilogue drops into fixed slots of this
                # head-pair's pipeline; PE slack in this ACT-bound loop is
                # filled with the next chunk's QKV (or, on the last chunk,
                # the Q1 partial out-proj).
                c = ch
                kmax = 4 * c + 4
                LAG = 2
                pend_copies = pend_norm = None
                filler = None
                # pull rate: the last chunk's filler (8 out-proj steps) is
                # small -- spread it across the whole pipeline instead of
                # exhausting it in the first head-pair
                spread = 1
                slot = 0
                for hp in range(HP):
                    if hp == 1:
                        if ch + 1 < C4:
                            emit_ln1(ch + 1)
                            filler = qkv_steps(ch + 1)
                            if ch == 1 and C4 == 4:
                                # Q0's partial out-proj is ready (chunk 0
                                # attention done) -- more PE filler for the
                                # ACT-bound pipeline
                                filler = itertools.chain(
                                    filler, outproj_steps(0, 0)
                                )
                        else:
                            # last chunk: fill with the Q1 partial out-proj
                            # (+ Q2 at full size, whose RS_a then launches
                            # right after the drain below)
                            filler = outproj_steps(1, 2)
                            if C4 == 4:
                                filler = itertools.chain(
                                    outproj_steps(2, 1), filler
                                )
                    avp = ps_av.tile([65, 1024], F32, tag="av", name="av")
                    av = [avp[:, 0:512], avp[:, 512:1024]]
                    pts = {}
                    for j in range(kmax + LAG):
                        if j < kmax:
                            i = j
                            # diagonal k-tiles only cover q >= k: shrink the
                            # score / exp / AV column range to the causally
                            # valid suffix.
                            r = i - 4 * c
                            qo = r * 128 if r > 0 else 0
                            pt = pt_pool.tile([128, 2, 512], BF16, tag="pt", name="pt")
                            pts[i] = (pt, qo)
                            for h01 in range(2):
                                po = h01 * 64
                                psc = ps_sc.tile([128, 512], F32, tag="sc", name="sc")
                                nc.tensor.matmul(
                                    psc[:, qo:512],
                                    lhsT=qkT[OT + hp][po : po + 64, i * 128 : (i + 1) * 128],
                                    rhs=qkT[hp][po : po + 64, c * 512 + qo : (c + 1) * 512],
                                    start=True,
                                    stop=True,
                                    tile_position=(po, 0),
                                )
                                nc.scalar.activation(
                                    pt[:, h01, qo:512], psc[:, qo:512], AF.Exp,
                                    bias=pbias[:, i : i + 1], scale=scale,
                                )
                            if r >= 0:
                                for h01 in range(2):
                                    sl = pt[:, h01, qo : qo + 128]
                                    nc.vector.tensor_mul(sl, sl, maskd[:])
                        if j == 1 and pend_copies is not None:
                            pend_copies()
                            pend_copies = None
                        if j == min(5, kmax + LAG - 1) and pend_norm is not None:
                            pend_norm()
                            pend_norm = None
                        if filler is not None:
                            slot += 1
                            if slot % spread == 0:
                                next(filler, None)
                        if j >= LAG:
                            i = j - LAG
                            pt, qo = pts.pop(i)
                            for h01 in range(2):
                                hloc = 2 * hp + h01
                                nc.tensor.matmul(
                                    av[h01][:, qo:512],
                                    lhsT=VO[i][:, hloc * 65 : (hloc + 1) * 65],
                                    rhs=pt[:, h01, qo:512],
                                    start=(i == 0),
                                    stop=(i == kmax - 1),
                                )
                    pend_copies, pend_norm = make_tail(hp, c, av)
                # chunk-end flush: out-proj consumers follow in program order
                pend_copies()
                pend_norm()
                if filler is not None:
                    for _ in filler:
                        pass
                def emit_rs_a():
                    if cfg.no_collective:
                        nc.sync.dma_start(rs_out[0][:], rs_in[0][0])
                        return
                    nc.gpsimd.collective_compute(
                        "ReduceScatter",
                        OP.add,
                        replica_groups=groups,
                        ins=[rs_in[0][:]],
                        outs=[rs_out[0][:]],
                    )

                if C4 == 4:
                    # Q0/Q2 out-projs ran as attention fillers (c1 / c3);
                    # RS_a launches as soon as the c3 fillers drain, still
                    # ~40us ahead of the MLP's first x2 use.
                    if ch == 3:
                        emit_rs_a()
                elif ch == (3 * S // 4 + 511) // 512 - 1:
                    outproj_quarter(0, 0)
                    outproj_quarter(2, 1)
                    emit_rs_a()
            outproj_quarter(3, 3)

        ph_stack.close()
        qk_vo_stack.close()

        def emit_rs_b():
            if cfg.no_collective:
                nc.sync.dma_start(rs_out[1][:], rs_in[1][0])
                return
            nc.gpsimd.collective_compute(
                "ReduceScatter",
                OP.add,
                replica_groups=groups,
                ins=[rs_in[1][:]],
                outs=[rs_out[1][:]],
            )

        # -------- LN2 + MLP (token-local, full FF) --------
        with (
            tc.tile_pool(name="x2h", bufs=1) as x2h_pool,
            tc.tile_pool(name="xn2", bufs=1) as xn2_pool,
            tc.tile_pool(name="ht", bufs=1) as ht_pool,
            tc.tile_pool(name="w1", bufs=3) as w1_pool,
            tc.tile_pool(name="w2", bufs=2) as w2_pool,
            tc.tile_pool(name="fin", bufs=2) as fin_pool,
            tc.tile_pool(name="ps_f1", bufs=2, space="PSUM") as ps_f1,
            tc.tile_pool(name="ps_f2", bufs=2, space="PSUM") as ps_f2,
            tc.tile_pool(name="ps_stat", bufs=1, space="PSUM") as ps_stat,
            tc.tile_pool(name="ps_bb", bufs=1, space="PSUM") as ps_bb,
            tc.tile_pool(name="sb_small", bufs=2) as sb_small,
            tc.tile_pool(name="sb_big", bufs=2) as sb_big,
            tc.tile_pool(name="sb_bc", bufs=2) as sb_bc,
        ):
            ln_pools = (ps_stat, ps_bb, sb_small, sb_big, sb_bc)
            bfc2 = w2_pool.tile([1, E], F32R, tag="bfc2", name="bfc2")
            nc.sync.dma_start(bfc2[:], ins["bfc2"])
            # x2 (exact residual stream for owned tokens) stays in SBUF for
            # LN2 stats, LN2 apply, and the final residual add. One tile row
            # per ReduceScatter chunk so the sc=0 MLP pass only depends on
            # the first collective.
            nq = cfg.SH // QW
            x2q = [
                [
                    x2h_pool.tile([128, QW], BF16, tag=f"x2h{g}_{et}", name=f"x2h{g}_{et}")
                    for et in range(ET)
                ]
                for g in range(nq)
            ]

            xn2_t = xn2_pool.tile([128, ET, cfg.SH], BF16, tag="xn2", name="xn2")
            xn2 = [xn2_t[:, et] for et in range(ET)]
            def load_x2q(g):
                # SWDGE (gpsimd) keeps the RS-dependent loads off the SP
                # HWDGE ring, whose FIFO also carries the fc1/fc2 weight
                # streams: a blocked collective wait at the SP queue head
                # would starve the MLP of weights (observed 31us PE stall).
                for et in range(ET):
                    nc.gpsimd.dma_start(
                        x2q[g][et][:], rs_out[g][et * 128 : (et + 1) * 128, :]
                    )

            assert CW % QW == 0
            gpc = CW // QW  # RS chunks per MLP chunk (1 at full size)
            FG = 4 if FT % 4 == 0 else 1  # ft-tiles per weight DMA
            EG = 2 if ET % 2 == 0 else 1  # et-tiles per weight DMA
            ht_of = {}

            def fc1(sc):
                ht = ht_pool.tile([128, FT, CW], BF16, tag="ht", name="ht")
                ht_of[sc] = ht
                for ft in range(FT):
                    if ft % FG == 0:
                        w1t = w1_pool.tile(
                            [128, ET, FG * 128], BF16, tag="w1", name="w1"
                        )
                        nc.sync.dma_start(
                            w1t[:],
                            ins["wfc1T"][
                                :, ft * 128 : (ft + FG) * 128
                            ].rearrange("(et p) f -> p et f", p=128),
                        )
                    fl = ft % FG
                    psum = ps_f1.tile([128, CW], F32, tag="f1", name="f1")
                    for et in range(ET):
                        nc.tensor.matmul(
                            psum[:],
                            lhsT=w1t[:, et, fl * 128 : (fl + 1) * 128],
                            rhs=xn2_t[:, et, sc * CW : (sc + 1) * CW],
                            start=(et == 0),
                            stop=(et == ET - 1),
                        )
                    if cfg.gelu_exact:
                        nc.scalar.activation(
                            ht[:, ft], psum[:], AF.Gelu,
                            bias=bfc1[:, ft : ft + 1], scale=1.0,
                        )
                    else:
                        tg = fin_pool.tile([128, CW], F32, tag="tg", name="tg")
                        nc.vector.tensor_scalar(
                            tg[:], psum[:], bfc1[:, ft : ft + 1], None, OP.add
                        )
                        sg = fin_pool.tile([128, CW], F32, tag="sg", name="sg")
                        nc.scalar.activation(sg[:], tg[:], AF.Sigmoid, scale=1.702)
                        nc.vector.tensor_mul(ht[:, ft], tg[:], sg[:])

            def fc2(sc):
                ht = ht_of.pop(sc)
                for et in range(ET):
                    if et % EG == 0:
                        w2t = w2_pool.tile(
                            [128, FT, EG * 128], BF16, tag="w2", name="w2"
                        )
                        nc.sync.dma_start(
                            w2t[:],
                            ins["wfc2T"][
                                :, et * 128 : (et + EG) * 128
                            ].rearrange("(ft p) e -> p ft e", p=128),
                        )
                    el = et % EG
                    fin = fin_pool.tile([128, CW], F32, tag="fin", name="fin")
                    psum = ps_f2.tile([128, CW], F32, tag="f2", name="f2")
                    nc.tensor.matmul(
                        psum[:],
                        lhsT=bfc2[:, et * 128 : (et + 1) * 128],
                        rhs=onesrow[:, 0:CW],
                        start=True,
                        stop=False,
                    )
                    for ft in range(FT):
                        nc.tensor.matmul(
                            psum[:],
                            lhsT=w2t[:, ft, el * 128 : (el + 1) * 128],
                            rhs=ht[:, ft, :],
                            start=False,
                            stop=(ft == FT - 1),
                        )
                    for gg in range(gpc):
                        nc.vector.tensor_add(
                            fin[:, gg * QW : (gg + 1) * QW],
                            psum[:, gg * QW : (gg + 1) * QW],
                            x2q[sc * gpc + gg][et][:],
                        )
                    nc.sync.dma_start(
                        outs["outT"][
                            et * 128 : (et + 1) * 128, sc * CW : (sc + 1) * CW
                        ],
                        fin[:],
                    )

            # Emission order matters: nothing on the SP/PE queues ahead of
            # the sc=0 pass may depend on RS_b, or the queue stalls. Full
            # size: LN2(1) slots between fc1(0) and fc2(0) -- by then the
            # (bf16) RS_b has landed, and its DVE chain hides under fc2(0).
            if gpc == 1 and nq > 1:
                load_x2q(0)
                ln_stats_apply(ln_pools, x2q[0], 0, eps2, xn2, 0, QW)
                emit_rs_b()
                with tc.tile_wait_until(cfg.rsb_wait_ms):
                    load_x2q(1)
                fc1(0)
                with tc.tile_wait_until(cfg.rsb_wait_ms):
                    ln_stats_apply(ln_pools, x2q[1], 0, eps2, xn2, QW, QW)
                fc2(0)
                for sc in range(1, SC):
                    fc1(sc)
                    fc2(sc)
            else:
                emit_rs_b()
                for g in range(nq):
                    load_x2q(g)
                for sc in range(SC):
                    for gg in range(gpc):
                        with tc.tile_wait_until(cfg.rsb_wait_ms, enable=sc > 0):
                            ln_stats_apply(
                                ln_pools, x2q[sc * gpc + gg], 0, eps2, xn2,
                                sc * CW + gg * QW, QW,
                            )
                    fc1(sc)
                    fc2(sc)


# ---------------------------------------------------------------------------
# host side
# ---------------------------------------------------------------------------

def prep_inputs(cfg: Cfg, x, td, ln1_g, ln1_b, ln2_g, ln2_b, w_qkv, b_qkv,
                w_o, b_o, w_fc1, b_fc1, w_fc2, b_fc2):
    """Build the per-core input maps (numpy, fp32)."""
    E, H, OL, HL = cfg.E, cfg.H, cfg.OL, cfg.HL
    import ml_dtypes

    f4 = np.float32
    asc = np.ascontiguousarray

    wq, wk, wv = w_qkv[0:E], w_qkv[E : 2 * E], w_qkv[2 * E : 3 * E]
    bq, bk, bvv = b_qkv[0:E], b_qkv[E : 2 * E], b_qkv[2 * E : 3 * E]

    shared = {}
    shared["onesrow"] = np.ones((1, 512), f4)
    oh = np.zeros((2, 128), f4)
    oh[0, 0:64] = 1.0
    oh[1, 64:128] = 1.0
    shared["onehot2"] = oh
    shared["ones128"] = np.ones((128, 8), f4)
    shared["ones128b"] = np.ones((128, 1), ml_dtypes.bfloat16)
    k_idx = np.arange(128)
    shared["maskd"] = asc((k_idx[:, None] <= k_idx[None, :]).astype(ml_dtypes.bfloat16))
    pb = np.zeros((128, cfg.ST), f4)
    for i in range(cfg.ST):
        kabs = i * 128 + k_idx
        pb[(kabs % td) == (td - 1), i] = NEG
    shared["pbias"] = pb
    shared["bfc2"] = asc(b_fc2[None, :].astype(f4))
    # MLP weights: full FF on every core (token-split MLP). bf16, not fp8:
    # e4m3 quantization of fc1/fc2 measures 1.6-2.3e-2 rel max-err on the
    # real inputs -- no margin against the 2e-2 gate.
    shared["wfc1T"] = asc((w_fc1 * ln2_g[None, :]).T.astype(ml_dtypes.bfloat16))  # [E, FF]
    shared["bfc1"] = asc(
        (b_fc1 + w_fc1 @ ln2_b).astype(f4).reshape(cfg.FT, 128).T
    )  # [128, FT]
    shared["wfc2T"] = asc(w_fc2.T.astype(ml_dtypes.bfloat16))  # [FF, E]

    per_tp = []
    for tp in range(2):
        o_sl = slice(tp * OL, (tp + 1) * OL)
        d = {}
        wqk = np.concatenate([wq[o_sl], wk[o_sl]], axis=0)  # [2*OL, E]
        d["wqkT"] = asc((wqk * ln1_g[None, :]).T.astype(ml_dtypes.bfloat16))  # [E, 2*OL]
        bqk_full = (
            np.concatenate([bq[o_sl], bk[o_sl]]) + wqk @ ln1_b
        ).astype(f4)  # [2*OL]
        d["bqk"] = asc(bqk_full.reshape(2 * cfg.OT, 128).T)  # [128, 2*OT]
        d["wvT"] = asc((wv[o_sl] * ln1_g[None, :]).T.astype(ml_dtypes.bfloat16))  # [E, OL]
        d["bv"] = asc((bvv[o_sl] + wv[o_sl] @ ln1_b)[None, :].astype(f4))  # [1, OL]
        d["woT"] = asc(w_o[:, o_sl].T.astype(ml_dtypes.bfloat16))  # [OL, E]
        per_tp.append(d)

    in_maps = []
    for c in range(2 * cfg.n_pairs):
        p, tp = c // 2, c % 2
        m = dict(shared)
        m.update(per_tp[tp])
        m["xq"] = asc(0.5 * x[p].T.astype(f4))  # [E, S]
        # residual-stream copy with the out-proj bias pre-folded (each pair
        # member contributes half of x and half of b_o; the ReduceScatter sum
        # reconstructs x + b_o + attn exactly)
        m["xqr"] = asc((0.5 * x[p].T + 0.5 * b_o[:, None]).astype(f4))  # [E, S]
        in_maps.append(m)
    return in_maps


_F32R_INPUTS = {
    "xq", "bv", "bfc2",
    "onesrow", "onehot2", "ones128",
}
_BF16_INPUTS = {"wfc2T", "woT", "maskd", "wfc1T", "wqkT", "wvT", "ones128b"}


def build_nc(cfg: Cfg, sample_map):
    _patch_act_tables()
    nc = bacc.Bacc(
        "TRN2", target_bir_lowering=False, debug=False,
        num_devices=2 * cfg.n_pairs,
    )
    ins = {}
    for name, arr in sample_map.items():
        if name in _BF16_INPUTS:
            dt_ = BF16
        elif name in _F32R_INPUTS:
            dt_ = F32R
        else:
            dt_ = F32
        ins[name] = nc.dram_tensor(
            name, list(arr.shape), dt_, kind="ExternalInput"
        ).ap()
    outs = {
        "outT": nc.dram_tensor(
            "outT", [cfg.E, cfg.SH], F32, kind="ExternalOutput"
        ).ap()
    }
    with tile.TileContext(nc) as tc:
        block_kernel(tc, cfg, ins, outs)
    nc.compile()
    return nc


_CACHE = {}


def _get_nc(cfg: Cfg, sample_map):
    if cfg not in _CACHE:
        _CACHE[cfg] = build_nc(cfg, sample_map)
    return _CACHE[cfg]


def assemble_output(cfg: Cfg, results):
    """results: list of per-core output dicts -> full [B, S, E]."""
    out = np.empty((cfg.B, cfg.S, cfg.E), np.float32)
    for p in range(cfg.n_pairs):
        out[p, 0 : cfg.SH] = results[2 * p]["outT"].T
        out[p, cfg.SH :] = results[2 * p + 1]["outT"].T
    return out


class Runner:
    """Cached PJRT runner (keeps the jitted executable and device-resident
    inputs so repeated calls don't re-trace / re-transfer)."""

    def __init__(self, nc, n_cores):
        import jax
        from jax.sharding import Mesh, PartitionSpec
        from jax.experimental.shard_map import shard_map
        from concourse import bass2jax, mybir as mb

        bass2jax.install_neuronx_cc_hook()
        self.nc = nc
        self.n_cores = n_cores
        partition_name = (
            nc.partition_id_tensor.name if nc.partition_id_tensor else None
        )
        in_names, out_names, out_avals, zero_outs = [], [], [], []
        for alloc in nc.m.functions[0].allocations:
            if not isinstance(alloc, mb.MemoryLocationSet):
                continue
            name = alloc.memorylocations[0].name
            if alloc.kind == "ExternalInput":
                if name != partition_name:
                    in_names.append(name)
            elif alloc.kind == "ExternalOutput":
                shape = tuple(alloc.tensor_shape)
                dtype = mb.dt.np(alloc.dtype)
                out_names.append(name)
                out_avals.append(jax.core.ShapedArray(shape, dtype))
                zero_outs.append(np.zeros(shape, dtype))
        self.in_names = list(in_names)
        self.out_names = out_names
        self.out_avals = out_avals
        self.zero_outs = zero_outs
        n_params = len(self.in_names)
        all_in = list(self.in_names) + list(out_names)
        if partition_name is not None:
            all_in.append(partition_name)
        donate = tuple(range(n_params, n_params + len(out_names)))

        def _body(*args):
            operands = list(args)
            if partition_name is not None:
                operands.append(bass2jax.partition_id_tensor())
            outs = bass2jax._bass_exec_p.bind(
                *operands,
                out_avals=tuple(out_avals),
                in_names=tuple(all_in),
                out_names=tuple(out_names),
                lowering_input_output_aliases=(),
                sim_require_finite=True,
                sim_require_nnan=True,
                nc=nc,
            )
            return tuple(outs)

        devices = jax.devices()[:n_cores]
        self.mesh = Mesh(np.asarray(devices), ("core",))
        in_specs = (PartitionSpec("core"),) * (n_params + len(out_names))
        out_specs = (PartitionSpec("core"),) * len(out_names)
        self.sharded = jax.jit(
            shard_map(
                _body, mesh=self.mesh, in_specs=in_specs, out_specs=out_specs,
                check_rep=False,
            ),
            donate_argnums=donate,
            keep_unused=True,
        )
        self._jax = jax

    def concat_inputs(self, in_maps):
        return [
            np.concatenate(
                [np.asarray(in_maps[c][n]) for c in range(self.n_cores)], axis=0
            )
            for n in self.in_names
        ]

    def fresh_zeros(self):
        return [
            np.zeros((self.n_cores * z.shape[0], *z.shape[1:]), z.dtype)
            for z in self.zero_outs
        ]

    def run(self, concat_in, zeros=None):
        if zeros is None:
            zeros = self.fresh_zeros()
        out_arrs = self.sharded(*concat_in, *zeros)
        return [
            {
                name: np.asarray(out_arrs[i]).reshape(
                    self.n_cores, *self.out_avals[i].shape
                )[c]
                for i, name in enumerate(self.out_names)
            }
            for c in range(self.n_cores)
        ]


_RUNNER = {}


def get_runner(cfg: Cfg, sample_map):
    if cfg not in _RUNNER:
        _RUNNER[cfg] = Runner(_get_nc(cfg, sample_map), 2 * cfg.n_pairs)
    return _RUNNER[cfg]


def make_in_maps(cfg: Cfg, inputs):
    x = np.asarray(inputs["x"], np.float32)
    td = int(np.asarray(inputs["transition_dim"]))
    return prep_inputs(
        cfg, x, td,
        np.asarray(inputs["ln1_g"], np.float32),
        np.asarray(inputs["ln1_b"], np.float32),
        np.asarray(inputs["ln2_g"], np.float32),
        np.asarray(inputs["ln2_b"], np.float32),
        np.asarray(inputs["w_qkv"], np.float32),
        np.asarray(inputs["b_qkv"], np.float32),
        np.asarray(inputs["w_o"], np.float32),
        np.asarray(inputs["b_o"], np.float32),
        np.asarray(inputs["w_fc1"], np.float32),
        np.asarray(inputs["b_fc1"], np.float32),
        np.asarray(inputs["w_fc2"], np.float32),
        np.asarray(inputs["b_fc2"], np.float32),
    )


def kernel(**inputs) -> np.ndarray:
    cfg = Cfg()
    in_maps = make_in_maps(cfg, inputs)
    runner = get_runner(cfg, in_maps[0])
    results = runner.run(runner.concat_inputs(in_maps))
    return assemble_output(cfg, results)



# revision 63
# speedup vs baseline: 1.0134x; 1.0134x over previous
"""Dense transformer block (nn_Block_87127706566879) on 8 TRN2 NeuronCores.

Sharding: DP over batch (4 pairs) x TP=2 within each pair.
Attention is head-split (8 of 16 heads per core, Megatron-style); the MLP
is TOKEN-split (each core owns half the sequence). The single collective
per core pair is a ReduceScatter after the attention out-proj: it sums
the two cores' partial out-proj contributions AND scatters the residual
stream by token ownership in one shot. The MLP then runs fully local on
the owned tokens (full FF width) and the block output needs no second
collective. The ReduceScatter is split into two token-quarter chunks so
the first can overlap the tail of attention + out-proj compute.

The residual stream lives TRANSPOSED on chip ([E, S], e on partitions).
LayerNorm stats (over e = partition dim) are computed with ones-vector
matmuls on the PE; per-s stats are broadcast back across partitions with
a K=1 ones matmul. LN gains/biases are folded into the following matmul
weights host-side; x is pre-scaled by 0.5 so the pair ReduceScatter
reconstructs x + attn exactly.

Matmuls run in float32r (fp32 storage, ~tf32 precision, full PE rate at
free-dim >= 256). Attention: scores are computed transposed
(S^T = [k, q], keys stationary, two heads row-packed via tile_position);
softmax exp runs on ACT with the periodic (k % td == td-1) mask folded
into the per-partition bias and the 1/sqrt(dh) scale folded into the
activation scale; causal masking of diagonal tiles is a 0/1 mask
multiply on the DVE. The AV matmul appends a ones column to V
([V | 1], M=65) so the softmax denominator accumulates for free in PSUM
row 64; normalization happens once on the (much smaller) attn output.
The MLP hidden activations are kept in bf16 (gelu writes bf16 directly)
so the full-FF hidden fits in SBUF; fc2 weights are bf16 as well.

Scheduling (what got HW exec from 919us to ~730us, neuron-profile):
- The attention inner loop is software-pipelined: AV matmuls lag the
  score matmuls by 2 k-tiles so the PE never blocks on the ACT exp, and
  each head-pair's epilogue is split into an early PSUM-release (copies
  of the unnormalized attn rows + raw denominators) dropped into the
  next head-pair's pipeline, plus a slack-tolerant normalization.
- The attention phase is ACT(exp)-bound, so the next chunk's LN1+QKV
  (and ready out-proj quarters) are emitted as per-slot fillers inside
  the attention pipeline -- the PE stream stays dense, which also keeps
  the HAM clock-gate at K=8/8 (cold-clock time fell 459us -> ~180us).
  A ~10us burst of dummy K=1 matmuls at kernel start pre-warms HAM
  during the initial weight DMAs.
- 1/x and rsqrt run on ACT as exp(-ln(x)) / exp(-0.5 ln(x)): ln+exp
  share one act-table set with the attention exp (bass's act-table
  chooser is patched accordingly), replacing 3.3us-each [1,512] DVE
  reciprocals and 40+ act-table thrashes.
- The ReduceScatters run in bf16 (residual error ~4e-3 << the 2e-2
  gate) and the RS-dependent x2 loads go through SWDGE (gpsimd) so a
  blocked collective wait never heads the SP HWDGE ring that streams
  the fc1/fc2 weights.
- fp8(e4m3) DoubleRow for the MLP was tried and REJECTED: it measures
  1.6-2.7e-2 rel max-err on the real inputs -- no margin at tol 2e-2.
"""

import itertools
import math
from contextlib import ExitStack
from dataclasses import dataclass

import numpy as np

import concourse.bass as bass
import concourse.tile as tile
from concourse import bacc, mybir
from concourse._compat import with_exitstack

F32 = mybir.dt.float32
F32R = mybir.dt.float32r
BF16 = mybir.dt.bfloat16
AF = mybir.ActivationFunctionType
OP = mybir.AluOpType
NEG = -1e30


_ACT_TABLES_PATCHED = False


def _patch_act_tables():
    """Restrict the act-table chooser to one exp+ln set (plus gelu/sigmoid).

    bacc's insert_act_table_loads picks the first act_info.json set that
    contains each function: Exp -> 'exp_and_others', Ln -> 'natural_log',
    which alternate every softmax-normalize (47 x 1.28us table loads on
    HW). Emptying all sets except 'natural_log_exp_and_others' (which has
    BOTH exp and ln), 'gelu_and_others', and 'sigmoid_and_others' forces a
    thrash-free choice; indices stay aligned with act_info.json.
    """
    global _ACT_TABLES_PATCHED
    if _ACT_TABLES_PATCHED:
        return
    _ACT_TABLES_PATCHED = True
    import functools
    import concourse.hw_specs as _hs
    import concourse.bacc as _bc

    orig = _hs.get_activation_tables
    keep = {"natural_log_exp_and_others", "gelu_and_others", "sigmoid_and_others"}

    @functools.cache
    def patched(arch):
        return {k: (v if k in keep else set()) for k, v in orig(arch).items()}

    _hs.get_activation_tables = patched
    _bc.get_activation_tables = patched


def act_fn(nc, out, in_, func, bias=0.0, scale=1.0):
    """nc.scalar.activation clone that also permits Reciprocal/Rsqrt.

    bass blocks these two on ACT for accuracy reasons; at this kernel's
    2e-2 tolerance the LUT error (~1e-3) is fine and it avoids the very
    slow [1, n] DVE reciprocal (3.3us each on HW)."""
    s = nc.scalar
    ins = [s.lower_ap(in_)]
    if func in (AF.Copy, AF.Reciprocal):
        bias_arg = mybir.ImmediateValue(dtype=mybir.dt.float32, value=float(bias))
    else:
        bias_arg = s.lower_ap(s.bass.const_aps.scalar_like(float(bias), in_))
    ins.append(bias_arg)
    ins.append(mybir.ImmediateValue(dtype=mybir.dt.float32, value=float(scale)))
    ins.append(mybir.ImmediateValue(dtype=mybir.dt.float32, value=0.0))
    return s.add_instruction(
        mybir.InstActivation(
            name=s.bass.get_next_instruction_name(),
            func=func,
            ins=ins,
            outs=[s.lower_ap(out)],
        )
    )


@dataclass(frozen=True)
class Cfg:
    B: int = 4
    S: int = 2048
    E: int = 1024
    H: int = 16
    FF: int = 4096
    n_pairs: int = 4  # cores = 2 * n_pairs
    reps: int = 1
    rsb_wait_ms: float = 0.5  # virtual-time hint for RS_b-dependent ops
    gelu_exact: bool = True  # False: x*sigmoid(1.702x) (CoreSim lacks Gelu)
    no_collective: bool = False  # replace RS with local DMA (timeline sim only)

    @property
    def Dh(self):
        return self.E // self.H

    @property
    def HL(self):
        return self.H // 2  # heads per core

    @property
    def HP(self):
        return self.HL // 2  # head pairs per core

    @property
    def OL(self):
        return self.HL * self.Dh  # attn out dims per core

    @property
    def ET(self):
        return self.E // 128

    @property
    def ST(self):
        return self.S // 128

    @property
    def SH(self):
        return self.S // 2  # tokens owned per core (MLP)

    @property
    def SC(self):
        return self.SH // 512 if self.SH >= 512 else 1  # 512-chunks of owned tokens

    @property
    def CW(self):
        return min(self.SH, 512)  # chunk width in MLP

    @property
    def QW(self):
        return self.S // 4  # token quarter (ReduceScatter granularity)

    @property
    def C4(self):
        return self.S // 512  # q-chunks (attention / phase-1)

    @property
    def FT(self):
        return self.FF // 128  # full-FF tiles (MLP is token-split)

    @property
    def OT(self):
        return self.OL // 128  # attn o-tiles (= head pairs)


@with_exitstack
def block_kernel(ctx: ExitStack, tc: tile.TileContext, cfg: Cfg, ins, outs):
    nc = tc.nc
    ET, SC, C4, FT, HP, OT = cfg.ET, cfg.SC, cfg.C4, cfg.FT, cfg.HP, cfg.OT
    S, E, OL, QW, CW = cfg.S, cfg.E, cfg.OL, cfg.QW, cfg.CW
    ST = cfg.ST
    eps1 = 1e-5 / 4.0  # LN1 runs on x/2
    eps2 = 1e-5       # LN2 runs on exact x2
    groups = [[2 * p, 2 * p + 1] for p in range(cfg.n_pairs)]

    cst = ctx.enter_context(tc.tile_pool(name="cst", bufs=1))

    def load_const(name, dt_):
        t = cst.tile(list(ins[name].shape), dt_, tag=name)
        nc.sync.dma_start(t[:], ins[name])
        return t

    onesrow = load_const("onesrow", F32R)      # [1, 512]
    onehot2 = load_const("onehot2", F32R)      # [2, 128]
    ones128 = load_const("ones128", F32R)      # [128, 8] (col 0: stats lhsT)
    ones128b = load_const("ones128b", BF16)    # [128, 1] bf16 stats lhsT
    maskd = load_const("maskd", BF16)          # [128, 128] 0/1 lower-tri (k<=q)
    pbias = load_const("pbias", F32)           # [128, ST] periodic -1e30 bias
    bqk = load_const("bqk", F32)               # [128, 2*OT]
    bv = load_const("bv", F32R)                # [1, OL]
    bfc1 = load_const("bfc1", F32)             # [128, FT]

    def ln_stats_apply(pools, x_tiles, sc_off, eps, out_tiles, out_off, n):
        """LN over the partition (e) dim for one n-col chunk of x^T."""
        ps_stat, ps_bc, sb_small, sb_big, sb_bc = pools
        stat = ps_stat.tile([64, n], F32, tag="stat", name="stat")
        psum_s = stat[0:1, :]
        psum_q = stat[32:33, :]
        for et in range(ET):
            xr = x_tiles[et][:, sc_off : sc_off + n]
            is16 = xr.dtype == BF16
            xr_f = xr if xr.dtype in (F32, BF16) else xr.bitcast(F32)
            if is16:
                xr_r = xr
            else:
                xr_r = xr if xr.dtype == F32R else xr.bitcast(F32R)
            sq = sb_big.tile([128, n], BF16, tag="sqt", name="sqt")
            nc.scalar.square(sq[:], xr_f)
            nc.tensor.matmul(
                psum_s[:],
                lhsT=(ones128b[:, 0:1] if is16 else ones128[:, 0:1]),
                rhs=xr_r,
                start=(et == 0), stop=(et == ET - 1),
            )
            nc.tensor.matmul(
                psum_q[:], lhsT=ones128b[:, 0:1], rhs=sq[:],
                start=(et == 0), stop=(et == ET - 1),
            )
        inv_e = 1.0 / cfg.E
        m = sb_small.tile([1, n], F32R, tag="m", name="m")
        nc.vector.tensor_scalar(m[:], psum_s[:], inv_e, None, OP.mult)
        var = sb_small.tile([1, n], F32, tag="var", name="var")
        nc.vector.tensor_scalar(var[:], psum_q[:], inv_e, eps, OP.mult, OP.add)
        t1 = sb_small.tile([1, n], F32, tag="t1", name="t1")
        nc.vector.tensor_mul(t1[:], m[:].bitcast(F32), m[:].bitcast(F32))
        nc.vector.tensor_sub(var[:], var[:], t1[:])
        # rstd = exp(-0.5 ln(var+eps)) on ACT: ln and exp share one act
        # table set (natural_log_exp_and_others) with the attention exp,
        # so this neither thrashes the 1.28us act tables nor pays the
        # 3.3us [1, n] DVE reciprocal.
        lg = sb_small.tile([1, n], F32, tag="lg", name="lg")
        act_fn(nc, lg[:], var[:], AF.Ln)
        rstd = sb_small.tile([1, n], F32R, tag="rstd", name="rstd")
        act_fn(nc, rstd[:], lg[:], AF.Exp, scale=-0.5)
        # broadcast m, rstd across partitions via K=1 ones matmul
        pm = ps_bc.tile([128, n], F32, tag="a", name="pm")
        nc.tensor.matmul(pm[:], lhsT=onesrow[:, 0:128], rhs=m[:], start=True, stop=True)
        m_b = sb_bc.tile([128, n], F32, tag="m_b", name="m_b")
        nc.vector.tensor_copy(m_b[:], pm[:])
        pv = ps_bc.tile([128, n], F32, tag="a", name="pv")
        nc.tensor.matmul(pv[:], lhsT=onesrow[:, 0:128], rhs=rstd[:], start=True, stop=True)
        r_b = sb_bc.tile([128, n], F32, tag="r_b", name="r_b")
        nc.vector.tensor_copy(r_b[:], pv[:])
        for et in range(ET):
            src_ap = x_tiles[et][:, sc_off : sc_off + n]
            src_f = src_ap if src_ap.dtype in (F32, BF16) else src_ap.bitcast(F32)
            out_ap = out_tiles[et][:, out_off : out_off + n]
            nc.vector.tensor_sub(out_ap, src_f, m_b[:])
            nc.vector.tensor_mul(out_ap, out_ap, r_b[:])

    for rep in range(cfg.reps):
        # -------- LN1 + QKV + attention (interleaved per 512-chunk) ------
        # QKV projection work is PE+DVE-heavy while attention is ACT-heavy
        # (softmax exp): interleaving the two per chunk overlaps them.
        # Attention for chunk c needs exactly the K/V tiles chunks <= c
        # produced. PSUM budget (8 banks): ps_a 2 (QK-proj / scores /
        # denominator-broadcast / out-proj, one rotating tag), ps_v 1,
        # ps_av 2, ps_stat 1 (sum at partition 0, sum-of-squares at 32),
        # ps_bb 2.
        rs_in = [
            nc.dram_tensor(f"rs_in{rep}_{g}", [2, E, QW], BF16) for g in range(2)
        ]
        rs_out = [
            nc.dram_tensor(f"rs_out{rep}_{g}", [E, QW], BF16) for g in range(2)
        ]
        qk_vo_stack = ExitStack()
        qk_pool = qk_vo_stack.enter_context(tc.tile_pool(name="qk", bufs=1))
        qkT = [qk_pool.tile([128, S], BF16, tag=f"qkT{i}", name=f"qkT{i}") for i in range(2 * OT)]
        vo_pool = qk_vo_stack.enter_context(tc.tile_pool(name="vo", bufs=1))
        VO = [vo_pool.tile([128, cfg.HL * 65], BF16, tag=f"vo{i}", name=f"vo{i}") for i in range(ST)]
        at_pool = qk_vo_stack.enter_context(tc.tile_pool(name="attnT", bufs=1))
        attnT = [at_pool.tile([128, S], BF16, tag=f"at{i}", name=f"at{i}") for i in range(OT)]
        ph_stack = ExitStack()
        _p = lambda *a, **k: ph_stack.enter_context(tc.tile_pool(*a, **k))
        xq_pool = _p(name="xq_sb", bufs=2)
        xn1_pool = _p(name="xn1_sb", bufs=2)
        wqk_pool = _p(name="wqk", bufs=1)
        wv_pool = _p(name="wv", bufs=1)
        wo_pool = _p(name="wo", bufs=1)
        pt_pool = _p(name="pt", bufs=3)
        den_pool = _p(name="den", bufs=2)
        xqs_pool = _p(name="xqs", bufs=2)
        arin_pool = _p(name="arin", bufs=2)
        # PSUM budget (8 banks): ps_a 2 (QKV / V / out-proj / K=1 broadcast,
        # one rotating tag), ps_sc 3 (bf16 score tiles, 1 bank each -> 3-deep
        # score lookahead ahead of exp), ps_av 2 (AV accumulator), ps_stat 1.
        ps_a = _p(name="ps_a", bufs=2, space="PSUM")
        ps_sc = _p(name="ps_sc", bufs=3, space="PSUM")
        ps_av = _p(name="ps_av", bufs=1, space="PSUM")
        ps_stat = _p(name="ps_stat", bufs=1, space="PSUM")
        sb_small = _p(name="sb_small", bufs=2)
        sb_big = _p(name="sb_big", bufs=2)
        sb_bc = _p(name="sb_bc", bufs=1)
        if True:
            ln_pools = (ps_stat, ps_a, sb_small, sb_big, sb_bc)
            scale = 1.0 / math.sqrt(cfg.Dh)
            # PE warm-up: ~10us of dummy K=1 matmuls (they only need the
            # tiny onesrow const) trip the HAM activity window to K=8/8
            # during the initial weight DMAs, so the first real matmuls run
            # at 2.4 GHz instead of paying the cold-clock ramp.
            for _ in range(30):
                wup = ps_sc.tile([128, 512], F32, tag="sc", name="wup")
                nc.tensor.matmul(
                    wup[:], lhsT=onesrow[:, 0:128], rhs=onesrow[:, 0:512],
                    start=True, stop=True,
                )
            # prefetch the first x chunk ahead of the (large) weight DMAs so
            # LN1 of chunk 0 starts immediately
            xq_first = [
                xq_pool.tile([128, 512], F32R, tag=f"xq{et}", name=f"xq{et}")
                for et in range(ET)
            ]
            for et in range(ET):
                nc.sync.dma_start(
                    xq_first[et][:], ins["xq"][et * 128 : (et + 1) * 128, 0:512]
                )
            wqk_sb = wqk_pool.tile([128, ET, 2 * OL], BF16, tag="wqk", name="wqk")
            nc.sync.dma_start(
                wqk_sb[:], ins["wqkT"].rearrange("(et p) o -> p et o", p=128)
            )
            wv_sb = wv_pool.tile([128, ET, OL], BF16, tag="wv", name="wv")
            nc.sync.dma_start(
                wv_sb[:], ins["wvT"].rearrange("(et p) o -> p et o", p=128)
            )
            wo_sb = wo_pool.tile([128, OT, E], BF16, tag="wo", name="wo")
            nc.sync.dma_start(wo_sb[:], ins["woT"].rearrange("(ot p) e -> p ot e", p=128))

            def outproj_steps(qq, slot):
                """Partial out-proj + (0.5x + 0.5bo) residual for quarter qq
                -> rs_in[slot] (the bias rides in xqr, host-folded), sliced
                per e-tile so it can fill attention-pipeline PE slack."""
                for et in range(ET):
                    w = min(QW, 512)
                    coff = qq * QW
                    psum = ps_a.tile([128, 512], F32, tag="a", name="o")
                    for ot in range(OT):
                        nc.tensor.matmul(
                            psum[:, 0:w],
                            lhsT=wo_sb[:, ot, et * 128 : (et + 1) * 128],
                            rhs=attnT[ot][:, coff : coff + w],
                            start=(ot == 0),
                            stop=(ot == OT - 1),
                        )
                    xqs = xqs_pool.tile([128, w], F32, tag="xqs", name="xqs")
                    nc.sync.dma_start(
                        xqs[:],
                        ins["xqr"][et * 128 : (et + 1) * 128, coff : coff + w],
                    )
                    arin = arin_pool.tile([128, w], BF16, tag="arin", name="arin")
                    nc.vector.tensor_add(arin[:], psum[:, 0:w], xqs[:])
                    g, member = slot // 2, slot % 2
                    nc.sync.dma_start(
                        rs_in[g][member, et * 128 : (et + 1) * 128, 0:w], arin[:]
                    )
                    yield

            def outproj_quarter(qq, slot):
                for _ in outproj_steps(qq, slot):
                    pass

            xq_of = {0: xq_first}
            xn1_of = {}

            def emit_ln1(ci):
                xn1c = [
                    xn1_pool.tile([128, 512], BF16, tag=f"xn1{et}", name=f"xn1{et}")
                    for et in range(ET)
                ]
                ln_stats_apply(ln_pools, xq_of.pop(ci), 0, eps1, xn1c, 0, 512)
                xn1_of[ci] = xn1c

            def make_tail(hp, c, av):
                """Per-head-pair epilogue, split so it can slot into the next
                head-pair's pipeline: `copies` frees the av PSUM bank (must
                run before the next AV chain starts); `norm` is the slack-
                tolerant 1/den = exp(-ln(den)) + broadcast + in-place mul."""
                dens = [
                    den_pool.tile([1, 512], F32, tag=f"den{h01}", name=f"den{h01}")
                    for h01 in range(2)
                ]

                def copies():
                    for h01 in range(2):
                        nc.vector.tensor_copy(dens[h01][:], av[h01][64:65, :])
                    for h01 in range(2):
                        nc.vector.tensor_copy(
                            attnT[hp][h01 * 64 : (h01 + 1) * 64, c * 512 : (c + 1) * 512],
                            av[h01][0:64, :],
                        )

                def norm():
                    for h01 in range(2):
                        lg = den_pool.tile([1, 512], F32, tag="lg", name="lg")
                        act_fn(nc, lg[:], dens[h01][:], AF.Ln)
                        rd = den_pool.tile([1, 512], F32R, tag="rd", name="rd")
                        with nc.allow_low_precision(reason="1/den f32r for matmul rhs"):
                            act_fn(nc, rd[:], lg[:], AF.Exp, scale=-1.0)
                        pbc = ps_a.tile([128, 512], F32, tag="a", name="bc")
                        nc.tensor.matmul(
                            pbc[:], lhsT=onesrow[:, 0:128], rhs=rd[:],
                            start=True, stop=True,
                        )
                        nc.vector.tensor_mul(
                            attnT[hp][h01 * 64 : (h01 + 1) * 64, c * 512 : (c + 1) * 512],
                            attnT[hp][h01 * 64 : (h01 + 1) * 64, c * 512 : (c + 1) * 512],
                            pbc[h01 * 64 : (h01 + 1) * 64, :],
                        )

                return copies, norm

            def qkv_steps(ci):
                """QKV projection for chunk ci, sliced into generator steps
                (~2-3 matmuls each) so it can interleave into the previous
                chunk's ACT-bound attention pipeline and keep the PE dense."""
                xn1c = xn1_of.pop(ci)
                co2 = ci * 512
                # Q,K projections: out qkT [o, s] (w stationary)
                for ot in range(2 * OT):
                    psum = ps_a.tile([128, 512], F32, tag="a", name="qk")
                    for et in range(ET):
                        nc.tensor.matmul(
                            psum[:],
                            lhsT=wqk_sb[:, et, ot * 128 : (ot + 1) * 128],
                            rhs=xn1c[et][:],
                            start=(et == 0),
                            stop=(et == ET - 1),
                        )
                        if et % 3 == 2:
                            yield
                    nc.vector.tensor_scalar(
                        qkT[ot][:, co2 : co2 + 512],
                        psum[:],
                        bqk[:, ot : ot + 1],
                        None,
                        OP.add,
                    )
                    yield
                # V projection: out V [s, o_v] (xn1 stationary), bias preloaded
                for stl in range(4):  # s-tiles within this 512-chunk
                    st = co2 // 128 + stl
                    psum = ps_a.tile([128, 512], F32, tag="a", name="v")[:, 0:OL]
                    nc.tensor.matmul(
                        psum[:, 0:OL], lhsT=onesrow[:, 0:128], rhs=bv[:],
                        start=True, stop=False,
                    )
                    for et in range(ET):
                        nc.tensor.matmul(
                            psum[:],
                            lhsT=xn1c[et][:, stl * 128 : (stl + 1) * 128],
                            rhs=wv_sb[:, et],
                            start=False,
                            stop=(et == ET - 1),
                        )
                        if et % 3 == 2:
                            yield
                    for h in range(cfg.HL):
                        nc.vector.tensor_copy(
                            VO[st][:, h * 65 : h * 65 + 64],
                            psum[:, h * 64 : (h + 1) * 64],
                        )
                    nc.vector.tensor_copy(
                        VO[st][:, 64 :: 65], ones128[:, 0 : cfg.HL]
                    )
                    yield

            emit_ln1(0)
            for _ in qkv_steps(0):
                pass
            if C4 > 1:
                xq_of[1] = [
                    xq_pool.tile([128, 512], F32R, tag=f"xq{et}", name=f"xq{et}")
                    for et in range(ET)
                ]
                for et in range(ET):
                    nc.sync.dma_start(
                        xq_of[1][et][:],
                        ins["xq"][et * 128 : (et + 1) * 128, 512:1024],
                    )
            for ch in range(C4):
                coff = ch * 512
                # prefetch x two chunks ahead (LN1(ch+1) is emitted mid-
                # attention below, so xq(ch+1) was fetched a chunk ago)
                if ch + 2 < C4:
                    xq_sb = [
                        xq_pool.tile([128, 512], F32R, tag=f"xq{et}", name=f"xq{et}")
                        for et in range(ET)
                    ]
                    for et in range(ET):
                        nc.sync.dma_start(
                            xq_sb[et][:],
                            ins["xq"][
                                et * 128 : (et + 1) * 128, coff + 1024 : coff + 1536
                            ],
                        )
                    xq_of[ch + 2] = xq_sb
                # ---- attention for q-chunk ch (software-pipelined) ----
                # The AV matmuls lag the score matmuls by LAG iterations so
                # the PE stream never waits on the ACT exp; the previous
                # head-pair's epilogue drops into fixed slots of this
                # head-pair's pipeline; PE slack in this ACT-bound loop is
                # filled with the next chunk's QKV (or, on the last chunk,
                # the Q1 partial out-proj).
                c = ch
                kmax = 4 * c + 4
                LAG = 2
                pend_copies = pend_norm = None
                filler = None
                # pull rate: the last chunk's filler (8 out-proj steps) is
                # small -- spread it across the whole pipeline instead of
                # exhausting it in the first head-pair
                spread = 1
                slot = 0
                for hp in range(HP):
                    if hp == 1:
                        if ch + 1 < C4:
                            emit_ln1(ch + 1)
                            filler = qkv_steps(ch + 1)
                            if ch == 1 and C4 == 4:
                                # Q0's partial out-proj is ready (chunk 0
                                # attention done) -- more PE filler for the
                                # ACT-bound pipeline
                                filler = itertools.chain(
                                    filler, outproj_steps(0, 0)
                                )
                        else:
                            # last chunk: fill with the Q1 partial out-proj
                            filler = outproj_steps(1, 2)
                    avp = ps_av.tile([65, 1024], F32, tag="av", name="av")
                    av = [avp[:, 0:512], avp[:, 512:1024]]
                    pts = {}
                    for j in range(kmax + LAG):
                        if j < kmax:
                            i = j
                            # diagonal k-tiles only cover q >= k: shrink the
                            # score / exp / AV column range to the causally
                            # valid suffix.
                            r = i - 4 * c
                            qo = r * 128 if r > 0 else 0
                            pt = pt_pool.tile([128, 2, 512], BF16, tag="pt", name="pt")
                            pts[i] = (pt, qo)
                            for h01 in range(2):
                                po = h01 * 64
                                psc = ps_sc.tile([128, 512], F32, tag="sc", name="sc")
                                nc.tensor.matmul(
                                    psc[:, qo:512],
                                    lhsT=qkT[OT + hp][po : po + 64, i * 128 : (i + 1) * 128],
                                    rhs=qkT[hp][po : po + 64, c * 512 + qo : (c + 1) * 512],
                                    start=True,
                                    stop=True,
                                    tile_position=(po, 0),
                                )
                                nc.scalar.activation(
                                    pt[:, h01, qo:512], psc[:, qo:512], AF.Exp,
                                    bias=pbias[:, i : i + 1], scale=scale,
                                )
                            if r >= 0:
                                for h01 in range(2):
                                    sl = pt[:, h01, qo : qo + 128]
                                    nc.vector.tensor_mul(sl, sl, maskd[:])
                        if j == 1 and pend_copies is not None:
                            pend_copies()
                            pend_copies = None
                        if j == min(5, kmax + LAG - 1) and pend_norm is not None:
                            pend_norm()
                            pend_norm = None
                        if filler is not None:
                            slot += 1
                            if slot % spread == 0:
                                next(filler, None)
                        if j >= LAG:
                            i = j - LAG
                            pt, qo = pts.pop(i)
                            for h01 in range(2):
                                hloc = 2 * hp + h01
                                nc.tensor.matmul(
                                    av[h01][:, qo:512],
                                    lhsT=VO[i][:, hloc * 65 : (hloc + 1) * 65],
                                    rhs=pt[:, h01, qo:512],
                                    start=(i == 0),
                                    stop=(i == kmax - 1),
                                )
                    pend_copies, pend_norm = make_tail(hp, c, av)
                # chunk-end flush: out-proj consumers follow in program order
                pend_copies()
                pend_norm()
                if filler is not None:
                    for _ in filler:
                        pass
                def emit_rs_a():
                    if cfg.no_collective:
                        nc.sync.dma_start(rs_out[0][:], rs_in[0][0])
                        return
                    nc.gpsimd.collective_compute(
                        "ReduceScatter",
                        OP.add,
                        replica_groups=groups,
                        ins=[rs_in[0][:]],
                        outs=[rs_out[0][:]],
                    )

                if ch == (3 * S // 4 + 511) // 512 - 1:
                    # Q0 and Q2 are attention-complete here: finish their
                    # out-proj (Q0 already ran as a c1 filler at full size)
                    # and launch RS_a so it lands well before the MLP's
                    # first x2 use.
                    if not (ch == 2 and C4 == 4):
                        outproj_quarter(0, 0)
                    outproj_quarter(2, 1)
                    emit_rs_a()
            outproj_quarter(3, 3)

        ph_stack.close()
        qk_vo_stack.close()

        def emit_rs_b():
            if cfg.no_collective:
                nc.sync.dma_start(rs_out[1][:], rs_in[1][0])
                return
            nc.gpsimd.collective_compute(
                "ReduceScatter",
                OP.add,
                replica_groups=groups,
                ins=[rs_in[1][:]],
                outs=[rs_out[1][:]],
            )

        # -------- LN2 + MLP (token-local, full FF) --------
        with (
            tc.tile_pool(name="x2h", bufs=1) as x2h_pool,
            tc.tile_pool(name="xn2", bufs=1) as xn2_pool,
            tc.tile_pool(name="ht", bufs=2) as ht_pool,
            tc.tile_pool(name="w1", bufs=3) as w1_pool,
            tc.tile_pool(name="w2", bufs=2) as w2_pool,
            tc.tile_pool(name="fin", bufs=2) as fin_pool,
            tc.tile_pool(name="ps_f1", bufs=2, space="PSUM") as ps_f1,
            tc.tile_pool(name="ps_f2", bufs=2, space="PSUM") as ps_f2,
            tc.tile_pool(name="ps_stat", bufs=1, space="PSUM") as ps_stat,
            tc.tile_pool(name="ps_bb", bufs=1, space="PSUM") as ps_bb,
            tc.tile_pool(name="sb_small", bufs=2) as sb_small,
            tc.tile_pool(name="sb_big", bufs=2) as sb_big,
            tc.tile_pool(name="sb_bc", bufs=2) as sb_bc,
        ):
            ln_pools = (ps_stat, ps_bb, sb_small, sb_big, sb_bc)
            bfc2 = w2_pool.tile([1, E], F32R, tag="bfc2", name="bfc2")
            nc.sync.dma_start(bfc2[:], ins["bfc2"])
            FG = 4 if FT % 4 == 0 else 1  # ft-tiles per weight DMA
            # prefetch fc1's first weight group at scope entry so fc1(0)
            # starts the moment LN2(0) lands instead of waiting on the DMA
            w1_first = w1_pool.tile([128, ET, FG * 128], BF16, tag="w1", name="w1")
            nc.sync.dma_start(
                w1_first[:],
                ins["wfc1T"][:, 0 : FG * 128].rearrange("(et p) f -> p et f", p=128),
            )
            # x2 (exact residual stream for owned tokens) stays in SBUF for
            # LN2 stats, LN2 apply, and the final residual add. One tile row
            # per ReduceScatter chunk so the sc=0 MLP pass only depends on
            # the first collective.
            nq = cfg.SH // QW
            x2q = [
                [
                    x2h_pool.tile([128, QW], BF16, tag=f"x2h{g}_{et}", name=f"x2h{g}_{et}")
                    for et in range(ET)
                ]
                for g in range(nq)
            ]

            xn2_t = xn2_pool.tile([128, ET, cfg.SH], BF16, tag="xn2", name="xn2")
            xn2 = [xn2_t[:, et] for et in range(ET)]
            def load_x2q(g):
                # SWDGE (gpsimd) keeps the RS-dependent loads off the SP
                # HWDGE ring, whose FIFO also carries the fc1/fc2 weight
                # streams: a blocked collective wait at the SP queue head
                # would starve the MLP of weights (observed 31us PE stall).
                for et in range(ET):
                    nc.gpsimd.dma_start(
                        x2q[g][et][:], rs_out[g][et * 128 : (et + 1) * 128, :]
                    )

            assert CW % QW == 0
            gpc = CW // QW  # RS chunks per MLP chunk (1 at full size)
            EG = 2 if ET % 2 == 0 else 1  # et-tiles per weight DMA
            ht_of = {}

            def fc1(sc):
                ht = ht_pool.tile([128, FT, CW], BF16, tag="ht", name="ht")
                ht_of[sc] = ht
                for ft in range(FT):
                    if ft % FG == 0:
                        if sc == 0 and ft == 0:
                            w1t = w1_first
                        else:
                            w1t = w1_pool.tile(
                                [128, ET, FG * 128], BF16, tag="w1", name="w1"
                            )
                            nc.sync.dma_start(
                                w1t[:],
                                ins["wfc1T"][
                                    :, ft * 128 : (ft + FG) * 128
                                ].rearrange("(et p) f -> p et f", p=128),
                            )
                    fl = ft % FG
                    psum = ps_f1.tile([128, CW], F32, tag="f1", name="f1")
                    for et in range(ET):
                        nc.tensor.matmul(
                            psum[:],
                            lhsT=w1t[:, et, fl * 128 : (fl + 1) * 128],
                            rhs=xn2_t[:, et, sc * CW : (sc + 1) * CW],
                            start=(et == 0),
                            stop=(et == ET - 1),
                        )
                    if cfg.gelu_exact:
                        nc.scalar.activation(
                            ht[:, ft], psum[:], AF.Gelu,
                            bias=bfc1[:, ft : ft + 1], scale=1.0,
                        )
                    else:
                        tg = fin_pool.tile([128, CW], F32, tag="tg", name="tg")
                        nc.vector.tensor_scalar(
                            tg[:], psum[:], bfc1[:, ft : ft + 1], None, OP.add
                        )
                        sg = fin_pool.tile([128, CW], F32, tag="sg", name="sg")
                        nc.scalar.activation(sg[:], tg[:], AF.Sigmoid, scale=1.702)
                        nc.vector.tensor_mul(ht[:, ft], tg[:], sg[:])

            def fc2(sc):
                ht = ht_of.pop(sc)
                for et in range(ET):
                    if et % EG == 0:
                        w2t = w2_pool.tile(
                            [128, FT, EG * 128], BF16, tag="w2", name="w2"
                        )
                        nc.sync.dma_start(
                            w2t[:],
                            ins["wfc2T"][
                                :, et * 128 : (et + EG) * 128
                            ].rearrange("(ft p) e -> p ft e", p=128),
                        )
                    el = et % EG
                    fin = fin_pool.tile([128, CW], F32, tag="fin", name="fin")
                    psum = ps_f2.tile([128, CW], F32, tag="f2", name="f2")
                    nc.tensor.matmul(
                        psum[:],
                        lhsT=bfc2[:, et * 128 : (et + 1) * 128],
                        rhs=onesrow[:, 0:CW],
                        start=True,
                        stop=False,
                    )
                    for ft in range(FT):
                        nc.tensor.matmul(
                            psum[:],
                            lhsT=w2t[:, ft, el * 128 : (el + 1) * 128],
                            rhs=ht[:, ft, :],
                            start=False,
                            stop=(ft == FT - 1),
                        )
                    for gg in range(gpc):
                        nc.vector.tensor_add(
                            fin[:, gg * QW : (gg + 1) * QW],
                            psum[:, gg * QW : (gg + 1) * QW],
                            x2q[sc * gpc + gg][et][:],
                        )
                    nc.sync.dma_start(
                        outs["outT"][
                            et * 128 : (et + 1) * 128, sc * CW : (sc + 1) * CW
                        ],
                        fin[:],
                    )

            # Emission order matters: nothing on the SP/PE queues ahead of
            # the sc=0 pass may depend on RS_b, or the queue stalls. Full
            # size: LN2(1) slots between fc1(0) and fc2(0) -- by then the
            # (bf16) RS_b has landed, and its DVE chain hides under fc2(0).
            if gpc == 1 and nq > 1:
                load_x2q(0)
                ln_stats_apply(ln_pools, x2q[0], 0, eps2, xn2, 0, QW)
                emit_rs_b()
                with tc.tile_wait_until(cfg.rsb_wait_ms):
                    load_x2q(1)
                fc1(0)
                with tc.tile_wait_until(cfg.rsb_wait_ms):
                    ln_stats_apply(ln_pools, x2q[1], 0, eps2, xn2, QW, QW)
                fc2(0)
                for sc in range(1, SC):
                    fc1(sc)
                    fc2(sc)
            else:
                emit_rs_b()
                for g in range(nq):
                    load_x2q(g)
                for sc in range(SC):
                    for gg in range(gpc):
                        with tc.tile_wait_until(cfg.rsb_wait_ms, enable=sc > 0):
                            ln_stats_apply(
                                ln_pools, x2q[sc * gpc + gg], 0, eps2, xn2,
                                sc * CW + gg * QW, QW,
                            )
                    fc1(sc)
                    fc2(sc)


# ---------------------------------------------------------------------------
# host side
# ---------------------------------------------------------------------------

def prep_inputs(cfg: Cfg, x, td, ln1_g, ln1_b, ln2_g, ln2_b, w_qkv, b_qkv,
                w_o, b_o, w_fc1, b_fc1, w_fc2, b_fc2):
    """Build the per-core input maps (numpy, fp32)."""
    E, H, OL, HL = cfg.E, cfg.H, cfg.OL, cfg.HL
    import ml_dtypes

    f4 = np.float32
    asc = np.ascontiguousarray

    wq, wk, wv = w_qkv[0:E], w_qkv[E : 2 * E], w_qkv[2 * E : 3 * E]
    bq, bk, bvv = b_qkv[0:E], b_qkv[E : 2 * E], b_qkv[2 * E : 3 * E]

    shared = {}
    shared["onesrow"] = np.ones((1, 512), f4)
    oh = np.zeros((2, 128), f4)
    oh[0, 0:64] = 1.0
    oh[1, 64:128] = 1.0
    shared["onehot2"] = oh
    shared["ones128"] = np.ones((128, 8), f4)
    shared["ones128b"] = np.ones((128, 1), ml_dtypes.bfloat16)
    k_idx = np.arange(128)
    shared["maskd"] = asc((k_idx[:, None] <= k_idx[None, :]).astype(ml_dtypes.bfloat16))
    pb = np.zeros((128, cfg.ST), f4)
    for i in range(cfg.ST):
        kabs = i * 128 + k_idx
        pb[(kabs % td) == (td - 1), i] = NEG
    shared["pbias"] = pb
    shared["bfc2"] = asc(b_fc2[None, :].astype(f4))
    # MLP weights: full FF on every core (token-split MLP). bf16, not fp8:
    # e4m3 quantization of fc1/fc2 measures 1.6-2.3e-2 rel max-err on the
    # real inputs -- no margin against the 2e-2 gate.
    shared["wfc1T"] = asc((w_fc1 * ln2_g[None, :]).T.astype(ml_dtypes.bfloat16))  # [E, FF]
    shared["bfc1"] = asc(
        (b_fc1 + w_fc1 @ ln2_b).astype(f4).reshape(cfg.FT, 128).T
    )  # [128, FT]
    shared["wfc2T"] = asc(w_fc2.T.astype(ml_dtypes.bfloat16))  # [FF, E]

    per_tp = []
    for tp in range(2):
        o_sl = slice(tp * OL, (tp + 1) * OL)
        d = {}
        wqk = np.concatenate([wq[o_sl], wk[o_sl]], axis=0)  # [2*OL, E]
        d["wqkT"] = asc((wqk * ln1_g[None, :]).T.astype(ml_dtypes.bfloat16))  # [E, 2*OL]
        bqk_full = (
            np.concatenate([bq[o_sl], bk[o_sl]]) + wqk @ ln1_b
        ).astype(f4)  # [2*OL]
        d["bqk"] = asc(bqk_full.reshape(2 * cfg.OT, 128).T)  # [128, 2*OT]
        d["wvT"] = asc((wv[o_sl] * ln1_g[None, :]).T.astype(ml_dtypes.bfloat16))  # [E, OL]
        d["bv"] = asc((bvv[o_sl] + wv[o_sl] @ ln1_b)[None, :].astype(f4))  # [1, OL]
        d["woT"] = asc(w_o[:, o_sl].T.astype(ml_dtypes.bfloat16))  # [OL, E]
        per_tp.append(d)

    in_maps = []
    for c in range(2 * cfg.n_pairs):
        p, tp = c // 2, c % 2
        m = dict(shared)
        m.update(per_tp[tp])
        m["xq"] = asc(0.5 * x[p].T.astype(f4))  # [E, S]
        # residual-stream copy with the out-proj bias pre-folded (each pair
        # member contributes half of x and half of b_o; the ReduceScatter sum
        # reconstructs x + b_o + attn exactly)
        m["xqr"] = asc((0.5 * x[p].T + 0.5 * b_o[:, None]).astype(f4))  # [E, S]
        in_maps.append(m)
    return in_maps


_F32R_INPUTS = {
    "xq", "bv", "bfc2",
    "onesrow", "onehot2", "ones128",
}
_BF16_INPUTS = {"wfc2T", "woT", "maskd", "wfc1T", "wqkT", "wvT", "ones128b"}


def build_nc(cfg: Cfg, sample_map):
    _patch_act_tables()
    nc = bacc.Bacc(
        "TRN2", target_bir_lowering=False, debug=False,
        num_devices=2 * cfg.n_pairs,
    )
    ins = {}
    for name, arr in sample_map.items():
        if name in _BF16_INPUTS:
            dt_ = BF16
        elif name in _F32R_INPUTS:
            dt_ = F32R
        else:
            dt_ = F32
        ins[name] = nc.dram_tensor(
            name, list(arr.shape), dt_, kind="ExternalInput"
        ).ap()
    outs = {
        "outT": nc.dram_tensor(
            "outT", [cfg.E, cfg.SH], F32, kind="ExternalOutput"
        ).ap()
    }
    with tile.TileContext(nc) as tc:
        block_kernel(tc, cfg, ins, outs)
    nc.compile()
    return nc


_CACHE = {}


def _get_nc(cfg: Cfg, sample_map):
    if cfg not in _CACHE:
        _CACHE[cfg] = build_nc(cfg, sample_map)
    return _CACHE[cfg]


def assemble_output(cfg: Cfg, results):
    """results: list of per-core output dicts -> full [B, S, E]."""
    out = np.empty((cfg.B, cfg.S, cfg.E), np.float32)
    for p in range(cfg.n_pairs):
        out[p, 0 : cfg.SH] = results[2 * p]["outT"].T
        out[p, cfg.SH :] = results[2 * p + 1]["outT"].T
    return out


class Runner:
    """Cached PJRT runner (keeps the jitted executable and device-resident
    inputs so repeated calls don't re-trace / re-transfer)."""

    def __init__(self, nc, n_cores):
        import jax
        from jax.sharding import Mesh, PartitionSpec
        from jax.experimental.shard_map import shard_map
        from concourse import bass2jax, mybir as mb

        bass2jax.install_neuronx_cc_hook()
        self.nc = nc
        self.n_cores = n_cores
        partition_name = (
            nc.partition_id_tensor.name if nc.partition_id_tensor else None
        )
        in_names, out_names, out_avals, zero_outs = [], [], [], []
        for alloc in nc.m.functions[0].allocations:
            if not isinstance(alloc, mb.MemoryLocationSet):
                continue
            name = alloc.memorylocations[0].name
            if alloc.kind == "ExternalInput":
                if name != partition_name:
                    in_names.append(name)
            elif alloc.kind == "ExternalOutput":
                shape = tuple(alloc.tensor_shape)
                dtype = mb.dt.np(alloc.dtype)
                out_names.append(name)
                out_avals.append(jax.core.ShapedArray(shape, dtype))
                zero_outs.append(np.zeros(shape, dtype))
        self.in_names = list(in_names)
        self.out_names = out_names
        self.out_avals = out_avals
        self.zero_outs = zero_outs
        n_params = len(self.in_names)
        all_in = list(self.in_names) + list(out_names)
        if partition_name is not None:
            all_in.append(partition_name)
        donate = tuple(range(n_params, n_params + len(out_names)))

        def _body(*args):
            operands = list(args)
            if partition_name is not None:
                operands.append(bass2jax.partition_id_tensor())
            outs = bass2jax._bass_exec_p.bind(
                *operands,
                out_avals=tuple(out_avals),
                in_names=tuple(all_in),
                out_names=tuple(out_names),
                lowering_input_output_aliases=(),
                sim_require_finite=True,
                sim_require_nnan=True,
                nc=nc,
            )
            return tuple(outs)

        devices = jax.devices()[:n_cores]
        self.mesh = Mesh(np.asarray(devices), ("core",))
        in_specs = (PartitionSpec("core"),) * (n_params + len(out_names))
        out_specs = (PartitionSpec("core"),) * len(out_names)
        self.sharded = jax.jit(
            shard_map(
                _body, mesh=self.mesh, in_specs=in_specs, out_specs=out_specs,
                check_rep=False,
            ),
            donate_argnums=donate,
            keep_unused=True,
        )
        self._jax = jax

    def concat_inputs(self, in_maps):
        return [
            np.concatenate(
                [np.asarray(in_maps[c][n]) for c in range(self.n_cores)], axis=0
            )
            for n in self.in_names
        ]

    def fresh_zeros(self):
        return [
            np.zeros((self.n_cores * z.shape[0], *z.shape[1:]), z.dtype)
            for z in self.zero_outs
        ]

    def run(self, concat_in, zeros=None):
        if zeros is None:
            zeros = self.fresh_zeros()
        out_arrs = self.sharded(*concat_in, *zeros)
        return [
            {
                name: np.asarray(out_arrs[i]).reshape(
                    self.n_cores, *self.out_avals[i].shape
                )[c]
                for i, name in enumerate(self.out_names)
            }
            for c in range(self.n_cores)
        ]


_RUNNER = {}


def get_runner(cfg: Cfg, sample_map):
    if cfg not in _RUNNER:
        _RUNNER[cfg] = Runner(_get_nc(cfg, sample_map), 2 * cfg.n_pairs)
    return _RUNNER[cfg]


def make_in_maps(cfg: Cfg, inputs):
    x = np.asarray(inputs["x"], np.float32)
    td = int(np.asarray(inputs["transition_dim"]))
    return prep_inputs(
        cfg, x, td,
        np.asarray(inputs["ln1_g"], np.float32),
        np.asarray(inputs["ln1_b"], np.float32),
        np.asarray(inputs["ln2_g"], np.float32),
        np.asarray(inputs["ln2_b"], np.float32),
        np.asarray(inputs["w_qkv"], np.float32),
        np.asarray(inputs["b_qkv"], np.float32),
        np.asarray(inputs["w_o"], np.float32),
        np.asarray(inputs["b_o"], np.float32),
        np.asarray(inputs["w_fc1"], np.float32),
        np.asarray(inputs["b_fc1"], np.float32),
        np.asarray(inputs["w_fc2"], np.float32),
        np.asarray(inputs["b_fc2"], np.float32),
    )


def kernel(**inputs) -> np.ndarray:
    cfg = Cfg()
    in_maps = make_in_maps(cfg, inputs)
    runner = get_runner(cfg, in_maps[0])
    results = runner.run(runner.concat_inputs(in_maps))
    return assemble_output(cfg, results)

